# revision 10
# baseline (speedup 1.0000x reference)
"""Trainium2 Bass kernel for masked dot-product attention (v2, bf16).

Problem: B=16, Lq=Lk=2048, d=128, fp32 in/out.
  scores = Q @ K^T / sqrt(d); mask key positions >= valid_len;
  attn = softmax(scores, axis=-1); out = attn @ V.

Sharding: 16 batches x 4 q-chunks of 512 = 64 shards, 8 per core, sorted by
key-tile count; slot s of every core runs the 8 shards ranked [8s, 8s+8) so
the compiled SPMD program bakes per-slot key extents E_s (rank-band maxima).

Design:
  * All matmul operands bf16 (host-rounded): halves input DMA bytes; PE still
    streams 1 column/cycle. num output bf16 (den stays f32).
  * exp in groups of 2 key tiles per activation ([128, 1024] PSUM tiles).
  * Denominator: each exp group is folded to one [128,512] partial on DVE
    (bf16 adds; the slot's final tile is z-gated via scalar_tensor_tensor),
    consecutive partials of a slot are combined pairwise, and ONE matmul per
    4 key tiles with a per-slot one-hot [128,8] selector stationary
    accumulates den into row s of a single persistent [8,512] PSUM tile,
    evacuated once at program end.
  * Masking: V rows at k >= valid_len are zeroed host-side (num exact); den's
    final key tile is gated by the per-partition 0/1 vector z. exp never
    sees a mask value.
  * Q^T and K^T ship in one merged [128, 512+T*128] DMA per slot (MM1 waits
    on a single transfer), V separately; all input DMAs issue up-front from
    the SP sequencer (HWDGE) into persistent SBUF tiles so transfers
    prefetch ahead of compute. Output DMAs issue from the otherwise-idle
    Pool/GpSimd sequencer (SWDGE) so they never block input prefetch of the
    next repeat iteration. Slot order: two small slots warm the pipeline
    while big transfers stream, then descending size, smallest last (short
    drain tail).
  * Flat software pipeline across slot boundaries: unit i = (slot, group);
    MM1+exp of unit i issue with MM2+fold of unit i-2 and den of unit i-3,
    giving ACT a full unit of slack so PE never waits on exp, including
    across slot edges. The Exp activation
    table is preloaded via a dummy activation during initial DMA.

Device program per unit (group of <=3 key tiles, 512-wide q-chunk):
  MM1:  S^T[k,q] = (K^T tile).T @ Q^T      (PE, bf16 -> PSUM f32, x2)
  exp:  E = exp(S^T / sqrt(d))             (ACT, PSUM -> SBUF bf16, group)
  MM2:  num^T[d,q] += V-tile contract E    (PE accumulate, x2)
  fold: P = e0 + e1 (z-gated on the slot's last tile)          (DVE, bf16)
  den:  pden[8,q] += sel_s[128,8].T @ (P_a + P_b)   (PE, one per 2 groups)
Host computes out = (num / den).T per shard.
"""

import math

import numpy as np

B, L, D = 16, 2048, 128
NCORES = 8
QCHUNK = 512
NQCHUNKS = L // QCHUNK
NSLOTS = B * NQCHUNKS // NCORES  # 8
GSZ = 3
SCALE = 1.0 / math.sqrt(D)

_programs = {}

_TRACE = False
_REPEAT = 1
_last_results = None


def _bf16(arr):
    import ml_dtypes

    return np.asarray(arr).astype(ml_dtypes.bfloat16)


def _slot_order(extents):
    """Two small warmup slots, then descending, smallest last."""
    idx = sorted(range(len(extents)), key=lambda s: -extents[s])
    warm = [idx[-3], idx[-2]]
    tail = [idx[-1]]
    mid = [s for s in idx if s not in warm and s not in tail]
    return warm + mid + tail


def _build_program(extents, repeat=1):
    import concourse.tile as tile
    from concourse import bacc, mybir

    F32 = mybir.dt.float32
    BF16 = mybir.dt.bfloat16
    ADD = mybir.AluOpType.add
    MULT = mybir.AluOpType.mult
    EXP = mybir.ActivationFunctionType.Exp

    order = _slot_order(extents)

    nc = bacc.Bacc("TRN2")

    ins = {}
    outs = {}
    for s, T in enumerate(extents):
        ins[f"qk{s}"] = nc.dram_tensor(
            f"qk{s}", [128, QCHUNK + T * 128], BF16, kind="ExternalInput"
        )
        ins[f"v{s}"] = nc.dram_tensor(f"v{s}", [128, T * 128], BF16, kind="ExternalInput")
        outs[f"num{s}"] = nc.dram_tensor(f"num{s}", [128, QCHUNK], BF16, kind="ExternalOutput")
    ins["zall"] = nc.dram_tensor("zall", [128, NSLOTS], F32, kind="ExternalInput")
    outs["den"] = nc.dram_tensor("den", [NSLOTS, QCHUNK], F32, kind="ExternalOutput")

    with tile.TileContext(nc) as tc:
        with (
            tc.tile_pool(name="const", bufs=1) as cpool,
            tc.tile_pool(name="inp", bufs=1) as inp,
            tc.tile_pool(name="epool", bufs=4) as epool,
            tc.tile_pool(name="ppool", bufs=8) as ppool,
            tc.tile_pool(name="opool", bufs=1) as opool,
            tc.tile_pool(name="ps_s", bufs=2, space="PSUM") as ps_s,
            tc.tile_pool(name="ps_o", bufs=1, space="PSUM") as ps_o,
            tc.tile_pool(name="ps_d", bufs=1, space="PSUM") as ps_d,
        ):
            # Per-slot one-hot selector stationaries: sel[:, 8s+s] = 1.
            sel = cpool.tile([128, NSLOTS * NSLOTS], BF16, tag="sel")
            nc.vector.memset(sel, 0.0)
            for s in range(NSLOTS):
                nc.vector.memset(sel[:, s * NSLOTS + s : s * NSLOTS + s + 1], 1.0)
            ztile = cpool.tile([128, NSLOTS], F32, tag="ztile")
            # Pull the Exp table load off the critical path: dummy activation
            # on a ready SBUF tile while input DMAs are still in flight.
            scratch = cpool.tile([128, 1], BF16, tag="scratch")
            nc.scalar.activation(scratch, sel[:, 0:1], EXP)

            # Persistent per-slot input tiles.
            qks, vts = [], []
            for s, T in enumerate(extents):
                qks.append(
                    inp.tile([128, QCHUNK + T * 128], BF16, tag=f"qk{s}", name=f"qk{s}")
                )
                vts.append(inp.tile([128, T * 128], BF16, tag=f"vt{s}", name=f"vt{s}"))

            # Persistent output staging (one buffer per slot: no recycling).
            osbs = [
                opool.tile([128, QCHUNK], BF16, tag=f"osb{s}", name=f"osb{s}")
                for s in range(NSLOTS)
            ]
            dsb = cpool.tile([NSLOTS, QCHUNK], F32, tag="dsb")

            for rep in range(repeat):
                nc.sync.dma_start(out=ztile, in_=ins["zall"][:, :])
                for s in order:
                    nc.sync.dma_start(out=qks[s], in_=ins[f"qk{s}"][:, :])
                    nc.sync.dma_start(out=vts[s], in_=ins[f"v{s}"][:, :])

                # Flat unit stream: (slot, tile-group of <=GSZ).
                units = []
                for s in order:
                    T = extents[s]
                    groups = [list(range(g, min(g + GSZ, T))) for g in range(0, T, GSZ)]
                    for g, gtiles in enumerate(groups):
                        units.append([s, g, gtiles, len(groups), None])

                nunits = len(units)
                # Pair consecutive units of a slot into den chunks: the
                # second unit's partial absorbs the first (one extra DVE
                # add), halving den matmuls. chunk_role: 0=defer, 1=combine
                # with previous, 2=single.
                chunk_role = []  # per unit: list of unit idxs to absorb (empty=defer), or None=defer
                pending_pp = []
                for u, (s, g, gtiles, ng, _) in enumerate(units):
                    last_of_slot = g == ng - 1
                    if len(pending_pp) == 1 or last_of_slot:
                        chunk_role.append(list(pending_pp))
                        pending_pp = []
                    else:
                        chunk_role.append(None)
                        pending_pp.append(u)
                den_idx = [u for u, r in enumerate(chunk_role) if r is not None]
                po = {}
                pden = ps_d.tile([NSLOTS, QCHUNK], F32, tag="pden")
                stage1 = {}  # unit idx -> eg tile
                for i in range(nunits + 3):
                    if i < nunits:
                        s, g, gtiles, ng, _ = units[i]
                        gn = len(gtiles)
                        # Timing variants (repeat>1) chain iterations through
                        # real data: MM1's moving operand for iteration r>0 is
                        # the previous iteration's num output (same [128,512]
                        # bf16 shape). Identical instruction stream and cost,
                        # but nothing is redundant, so no compiler layer can
                        # elide the repeated work. Iteration 0 (the graded
                        # path) always reads the DMA'd Q^T.
                        qt = qks[s][:, :QCHUNK] if rep == 0 else osbs[s]
                        pss = ps_s.tile([128, GSZ * QCHUNK], F32, tag="ps")
                        for j, t in enumerate(gtiles):
                            nc.tensor.matmul(
                                pss[:, j * QCHUNK : (j + 1) * QCHUNK],
                                qks[s][:, QCHUNK + t * 128 : QCHUNK + (t + 1) * 128],
                                qt,
                                start=True,
                                stop=True,
                            )
                        eg = epool.tile([128, GSZ * QCHUNK], BF16, tag="eg")
                        nc.scalar.activation(
                            eg[:, : gn * QCHUNK],
                            pss[:, : gn * QCHUNK],
                            EXP,
                            # Iterations r>0 of timing variants feed num back
                            # as Q; the tiny scale keeps the chained values
                            # bounded (identical instruction cost).
                            scale=SCALE if rep == 0 else SCALE / 64.0,
                        )
                        stage1[i] = eg

                    if 0 <= i - 2 < nunits:
                        s, g, gtiles, ng, _ = units[i - 2]
                        T = extents[s]
                        eg = stage1.pop(i - 2)
                        if g == 0:
                            po[s] = ps_o.tile([128, QCHUNK], F32, tag="po", name=f"po{s}")
                        for j, t in enumerate(gtiles):
                            nc.tensor.matmul(
                                po[s],
                                vts[s][:, t * 128 : (t + 1) * 128],
                                eg[:, j * QCHUNK : (j + 1) * QCHUNK],
                                start=(t == 0),
                                stop=(t == T - 1),
                            )
                        # Fold the group to one [128,512] partial (bf16); the
                        # slot's final tile is gated by z.
                        pp = ppool.tile([128, QCHUNK], BF16, tag="pp")
                        z = ztile[:, s : s + 1]
                        is_last = gtiles[-1] == T - 1
                        es = [eg[:, j * QCHUNK : (j + 1) * QCHUNK] for j in range(len(gtiles))]
                        if len(es) == 1:
                            if is_last:
                                nc.vector.tensor_scalar_mul(pp, es[0], z)
                            else:
                                nc.vector.tensor_copy(pp, es[0])
                        else:
                            if is_last:
                                nc.vector.scalar_tensor_tensor(
                                    pp, es[-1], z, es[-2], op0=MULT, op1=ADD
                                )
                            else:
                                nc.vector.tensor_tensor(pp, es[-2], es[-1], op=ADD)
                            for e in es[:-2]:
                                nc.vector.tensor_tensor(pp, pp, e, op=ADD)
                        units[i - 2][4] = pp
                        if g == ng - 1:
                            # Slot's accumulation is complete: evacuate num.
                            nc.vector.tensor_copy(osbs[s], po[s])
                            nc.gpsimd.dma_start(out=outs[f"num{s}"][:, :], in_=osbs[s])

                    if i - 3 >= 0:
                        u = i - 3
                        s, g, gtiles, ng, pp = units[u]
                        role = chunk_role[u]
                        if role is not None:
                            for up in role:
                                nc.vector.tensor_tensor(pp, pp, units[up][4], op=ADD)
                            nc.tensor.matmul(
                                pden,
                                sel[:, s * NSLOTS : (s + 1) * NSLOTS],
                                pp,
                                start=(u == den_idx[0]),
                                stop=(u == den_idx[-1]),
                            )
                nc.vector.tensor_copy(dsb, pden)
                nc.scalar.dma_start(out=outs["den"][:, :], in_=dsb)

    nc.finalize()
    return nc


def _get_program(extents, repeat=1):
    key = (tuple(extents), repeat)
    if key not in _programs:
        _programs[key] = _build_program(tuple(extents), repeat)
    return _programs[key]


def _shard_plan(vl):
    """64 (batch, q-chunk) shards sorted by key-tile count desc; slot s of
    core c runs shard rank s*8+c. Returns (shards, extents)."""
    tiles = [max(1, int(math.ceil(int(vl[b]) / 128.0))) for b in range(B)]
    shards = sorted(
        ((tiles[b], b, qc) for b in range(B) for qc in range(NQCHUNKS)),
        key=lambda x: (-x[0], x[1], x[2]),
    )
    extents = tuple(shards[s * NCORES][0] for s in range(NSLOTS))
    return shards, extents


def _make_in_maps(queries, keys, values, vl, shards, extents):
    # kt/vt depend only on (batch, extent): memoize across the 4 q-shards
    kcache = {}

    def kv(b, T):
        key = (b, T)
        if key not in kcache:
            n = int(vl[b])
            vs = values[b, : T * 128].copy()
            vs[n:] = 0.0
            # Masked K columns are zeroed: their scores become exactly 0 and
            # exp gives exactly 1.0, so the ungated padding-tile den excess
            # is the known constant max(0, (T-1)*128 - n), subtracted on the
            # host after readback (the slot's last tile is z-gated on
            # device). V rows are zeroed so num is exact either way.
            ks = keys[b, : T * 128].copy()
            ks[n:] = 0.0
            kcache[key] = (
                np.ascontiguousarray(_bf16(ks.T)),
                np.ascontiguousarray(
                    _bf16(vs.reshape(T, 128, D).transpose(1, 0, 2).reshape(128, T * D))
                ),
            )
        return kcache[key]

    qtr = {}

    def qtb(b):
        if b not in qtr:
            qtr[b] = _bf16(queries[b].T)
        return qtr[b]

    import ml_dtypes

    in_maps = [{} for _ in range(NCORES)]
    for s in range(NSLOTS):
        T = extents[s]
        for c in range(NCORES):
            _, b, qc = shards[s * NCORES + c]
            kt, vt = kv(b, T)
            n = int(vl[b])
            m = in_maps[c]
            if "zall" not in m:
                m["zall"] = np.zeros((128, NSLOTS), np.float32)
            nlast = max(0, n - (T - 1) * 128)  # valid rows in last tile
            if nlast > 0:
                m["zall"][:nlast, s] = 1.0
            qk = np.empty((128, QCHUNK + T * 128), ml_dtypes.bfloat16)
            qk[:, :QCHUNK] = qtb(b)[:, qc * QCHUNK : (qc + 1) * QCHUNK]
            qk[:, QCHUNK:] = kt
            m[f"qk{s}"] = qk
            m[f"v{s}"] = vt
    return in_maps


def kernel(queries, keys, values, valid_lens):
    from concourse.bass_utils import run_bass_kernel_spmd

    queries = np.ascontiguousarray(queries, dtype=np.float32)
    keys = np.ascontiguousarray(keys, dtype=np.float32)
    values = np.ascontiguousarray(values, dtype=np.float32)
    vl = np.asarray(valid_lens).astype(np.int64).clip(1, L)
    assert queries.shape == (B, L, D), queries.shape

    shards, extents = _shard_plan(vl)
    nc = _get_program(extents, _REPEAT)
    in_maps = _make_in_maps(queries, keys, values, vl, shards, extents)

    res = run_bass_kernel_spmd(nc, in_maps, core_ids=list(range(NCORES)), trace=_TRACE)
    globals()["_last_results"] = res

    out = np.empty((B, L, D), np.float32)
    for s in range(NSLOTS):
        for c in range(NCORES):
            _, b, qc = shards[s * NCORES + c]
            r = res.results[c]
            num = np.asarray(r[f"num{s}"]).astype(np.float32)  # [128, QCHUNK]
            n = int(vl[b])
            excess = max(0, (s_T - 1) * 128 - n) if (s_T := extents[s]) else 0
            den = np.asarray(r["den"])[s : s + 1, :] - excess  # [1, QCHUNK]
            out[b, qc * QCHUNK : (qc + 1) * QCHUNK] = (num / den).T
    return out
